# revision 35
# baseline (speedup 1.0000x reference)
"""Trainium2 Bass kernel for nn_AttentionAggregator (gnn_message_passing).

Two SPMD launches over 8 NeuronCores, data-parallel over nodes (512 users +
512 items per core), with a tiny host relay between them.

Key ideas:
  - Algebraic reorder: relu(softmax(Q K^T) @ C @ W) == relu(softmax(Q K^T) @ (C @ W)),
    shrinking the dominant matmul from [4096,4096]@[4096,2048] to
    [4096,4096]@[4096,128] (~9x fewer FLOPs).
  - Launch 1 runs the data-dependent row gathers on GPSIMD (the serial cost
    there is SWDGE descriptor/ring throughput, so gathers are issued as small
    sub-calls rotated over 4 SWDGE queues to drain concurrently at ~225 GB/s)
    while every other engine computes underneath that shadow:
      * review rows arrive as 1KB 4-row blocks via the custom dma_gather
        (block id r//4 fits int16 -- no windowing/sorting); a predicated
        4-way DVE select picks row r%4 per entry;
      * item/user rows (4096-row tables) gather directly with int16 ids;
      * gathered tiles are PE-transposed in slot pairs and projected against
        host-restacked bf16 weight blocks (C @ W) -> h blocks [512,128];
      * concurrently S^T = K q^T runs on the tensor engine (fp16 V^T,
        two K=64 matmuls packed into disjoint PE row groups) and exp(S/8)
        on ScalarE (scores ~N(0,1): no max subtraction), emitting E^T in
        bf16 to DRAM.
  - Launch 2: PV matmul with bf16 E^T stationary (fast weight load) against
    [h | 1] bf16 (fused row-sum column), reciprocal-normalize + relu.
"""

import sys

for _p in ("/opt/trn_rl_repo",):
    if _p not in sys.path:
        sys.path.append(_p)

import numpy as np

import concourse.bacc as bacc
import concourse.mybir as mybir
import concourse.tile as tile
from concourse.bass_utils import run_bass_kernel_spmd
from concourse.masks import make_identity

F32 = mybir.dt.float32
BF16 = mybir.dt.bfloat16
FP16 = mybir.dt.float16
I16 = mybir.dt.int16
I8 = mybir.dt.int8
AF = mybir.ActivationFunctionType
MULT = mybir.AluOpType.mult
ADD = mybir.AluOpType.add

N_REV, NU, DEG, D, HID = 100000, 4096, 16, 64, 128
N_CORES = 8
UB = NU // N_CORES          # 512 rows per core per side
NT = UB // 128              # 4 user tiles per core
NSLOT = NT * DEG            # 64 gathered slots per side (c = t*16 + j)
NG = UB * DEG               # 8192 gathered rows per table per side
MT = NU // 128              # 32 m tiles
QB = UB
QT = QB // 128
G = 2                       # m-tiles per QK/exp group
BLK = 4                     # review rows per gathered block
SCALE = 1.0 / float(np.sqrt(D))


def _build_k1():
    nc = bacc.Bacc("TRN2", target_bir_lowering=False, debug=False,
                   enable_asserts=True, num_devices=N_CORES,
                   num_swdge_queues=4)
    rev = nc.dram_tensor("rev", [N_REV, D], F32, kind="ExternalInput")
    usert = nc.dram_tensor("usert", [NU, D], F32, kind="ExternalInput")
    itemt = nc.dram_tensor("itemt", [NU, D], F32, kind="ExternalInput")
    wa_u = nc.dram_tensor("wa_u", [DEG * D, HID], BF16, kind="ExternalInput")
    wb_u = nc.dram_tensor("wb_u", [DEG * D, HID], BF16, kind="ExternalInput")
    wa_i = nc.dram_tensor("wa_i", [DEG * D, HID], BF16, kind="ExternalInput")
    wb_i = nc.dram_tensor("wb_i", [DEG * D, HID], BF16, kind="ExternalInput")
    bidx = nc.dram_tensor("bidx", [2, 128, NG // 16], I16, kind="ExternalInput")
    selm = nc.dram_tensor("selm", [2, 128, NSLOT, BLK], I8, kind="ExternalInput")
    iidx = nc.dram_tensor("iidx", [2, 128, NG // 16], I16, kind="ExternalInput")
    vtu = nc.dram_tensor("vtu", [2 * D, NU], FP16, kind="ExternalInput")
    vtuq = nc.dram_tensor("vtuq", [2 * D, QB], FP16, kind="ExternalInput")
    vti = nc.dram_tensor("vti", [2 * D, NU], FP16, kind="ExternalInput")
    vtiq = nc.dram_tensor("vtiq", [2 * D, QB], FP16, kind="ExternalInput")
    hu = nc.dram_tensor("hu", [UB, HID], F32, kind="ExternalOutput")
    hi = nc.dram_tensor("hi", [UB, HID], F32, kind="ExternalOutput")
    et = nc.dram_tensor("et", [2, MT, 128, QB], BF16, kind="ExternalOutput")

    with tile.TileContext(nc) as tc:
        with (
            tc.tile_pool(name="singles", bufs=1) as singles,
            tc.tile_pool(name="stgp", bufs=4) as stgp,
            tc.tile_pool(name="xp", bufs=3) as xp,
            tc.tile_pool(name="xtp", bufs=2) as xtp,
            tc.tile_pool(name="outb", bufs=4) as outb,
            tc.tile_pool(name="vtp", bufs=1) as vtp,
            tc.tile_pool(name="etp", bufs=3) as etp,
            tc.tile_pool(name="sps", bufs=2, space="PSUM") as sps,
            tc.tile_pool(name="tps", bufs=3, space="PSUM") as tps,
            tc.tile_pool(name="hps", bufs=1, space="PSUM") as hps,
        ):
            bidx_sb = singles.tile([128, 2, NG // 16], I16)
            nc.sync.dma_start(out=bidx_sb[:], in_=bidx.ap().rearrange("a p s -> p a s"))
            iidx_sb = singles.tile([128, 2, NG // 16], I16)
            nc.sync.dma_start(out=iidx_sb[:], in_=iidx.ap().rearrange("a p s -> p a s"))
            selm_sb = singles.tile([128, 2, NSLOT, BLK], I8)
            nc.sync.dma_start(out=selm_sb[:], in_=selm.ap().rearrange("a p c b -> p a c b"))
            ident = singles.tile([128, 128], F32)
            make_identity(nc, ident[:])
            w_sb = {}
            for nm, t in (("wa_u", wa_u), ("wb_u", wb_u), ("wa_i", wa_i), ("wb_i", wb_i)):
                w = singles.tile([128, DEG // 2, HID], BF16, tag=nm, name=f"{nm}_sb")
                nc.sync.dma_start(out=w[:], in_=t.ap().rearrange("(q k) n -> k q n", q=DEG // 2))
                w_sb[nm] = w

            qctr = [0]
            for side, (itbl, vt_d, vtq_d, wa, wb, hout) in enumerate((
                (itemt, vtu, vtuq, "wa_u", "wb_u", hu),
                (usert, vti, vtiq, "wa_i", "wb_i", hi),
            )):
                # ---- dense scores + exp (runs under the gather shadow) ----
                vt_sb = vtp.tile([2 * D, NU], FP16, tag="vt")
                vtq_sb = vtp.tile([2 * D, QB], FP16, tag="vtq")
                nc.sync.dma_start(out=vt_sb[:], in_=vt_d.ap())
                nc.sync.dma_start(out=vtq_sb[:], in_=vtq_d.ap())
                for g in range(MT // G):
                    s_ps = sps.tile([128, G, QB], F32, tag="sps")
                    for k in range(G):
                        m = g * G + k
                        lo = (k % 2) * D
                        nc.tensor.matmul(
                            s_ps[:, k, :],
                            lhsT=vt_sb[lo:lo + D, m * 128:(m + 1) * 128],
                            rhs=vtq_sb[lo:lo + D, :],
                            start=True, stop=True,
                            tile_position=(lo, 0),
                        )
                    etb = etp.tile([128, G, QB], BF16, tag="etb")
                    nc.scalar.activation(etb[:], s_ps[:], AF.Exp, scale=SCALE)
                    nc.sync.dma_start(
                        out=et.ap()[side, g * G:(g + 1) * G, :, :].rearrange("g p q -> p g q"),
                        in_=etb[:],
                    )

                # ---- per-tile gathers rotated across the 4 SWDGE queues;
                # select + transpose + project pipelined right behind each tile
                rev_blk = rev.ap().rearrange("(n b) e -> n (b e)", b=BLK)
                for t in range(NT):
                    sl = slice(t * DEG, (t + 1) * DEG)
                    TG = NG // NT    # 2048 rows per tile
                    t0 = t * TG      # first entry of this tile
                    # small sub-calls rotated over all 4 SWDGE queues drain
                    # concurrently (~226 GB/s vs ~105 single-queue)
                    stg = stgp.tile([128, DEG, BLK * D], F32, tag="stg")
                    CH = 512
                    for s in range(TG // CH):
                        e0 = t0 + s * CH
                        nc.gpsimd.dma_gather(
                            out_ap=stg[:, s * (CH // 128) : (s + 1) * (CH // 128), :],
                            in_ap=rev_blk,
                            idxs_ap=bidx_sb[:, side, e0 // 16:(e0 + CH) // 16],
                            num_idxs=CH, num_idxs_reg=CH, elem_size=BLK * D,
                            single_packet=False, queue_num=qctr[0] % 4,
                        )
                        qctr[0] += 1
                    XI = xp.tile([128, DEG, D], F32, tag="XI")
                    CHI = 512
                    for s in range(TG // CHI):
                        e0 = t0 + s * CHI
                        nc.gpsimd.dma_gather(
                            out_ap=XI[:, s * (CHI // 128) : (s + 1) * (CHI // 128), :],
                            in_ap=itbl.ap(),
                            idxs_ap=iidx_sb[:, side, e0 // 16:(e0 + CHI) // 16],
                            num_idxs=CHI, num_idxs_reg=CHI, elem_size=D,
                            single_packet=False, queue_num=qctr[0] % 4,
                        )
                        qctr[0] += 1
                    XR = xp.tile([128, DEG, D], F32, tag="XR")
                    nc.vector.tensor_copy(XR[:], stg[:, :, 0:D])
                    for b in range(1, BLK):
                        mb = selm_sb[:, side, sl, b][:, :, None].broadcast_to([128, DEG, D])
                        nc.vector.copy_predicated(XR[:], mb, stg[:, :, b * D:(b + 1) * D])

                    XT = xtp.tile([128, DEG, 128], BF16, tag="XT")
                    for q in range(DEG // 2):
                        ps_r = tps.tile([128, 128], F32, tag="tps", name=f"tr{side}_{t}_{q}")
                        nc.tensor.transpose(ps_r[:], XR[:, 2 * q: 2 * q + 2, :], ident[:])
                        nc.vector.tensor_copy(XT[:, q, :], ps_r[:])
                        ps_i = tps.tile([128, 128], F32, tag="tps", name=f"ti{side}_{t}_{q}")
                        nc.tensor.transpose(ps_i[:], XI[:, 2 * q: 2 * q + 2, :], ident[:])
                        nc.vector.tensor_copy(XT[:, DEG // 2 + q, :], ps_i[:])
                    h_ps = hps.tile([128, HID], F32, tag="hps")
                    for q in range(DEG // 2):
                        nc.tensor.matmul(h_ps[:], lhsT=XT[:, q, :], rhs=w_sb[wa][:, q, :],
                                         start=(q == 0), stop=False, skip_group_check=True)
                        nc.tensor.matmul(h_ps[:], lhsT=XT[:, DEG // 2 + q, :], rhs=w_sb[wb][:, q, :],
                                         start=False, stop=(q == DEG // 2 - 1), skip_group_check=True)
                    h_sb = outb.tile([128, HID], F32, tag="hsb")
                    nc.vector.tensor_copy(h_sb[:], h_ps[:])
                    nc.sync.dma_start(out=hout.ap()[t * 128:(t + 1) * 128, :], in_=h_sb[:])

    nc.compile()
    return nc


def _build_k2():
    nc = bacc.Bacc("TRN2", target_bir_lowering=False, debug=False,
                   enable_asserts=True, num_devices=N_CORES)
    et = nc.dram_tensor("et", [2, MT, 128, QB], BF16, kind="ExternalInput")
    hau = nc.dram_tensor("hau", [128, MT, HID + 1], BF16, kind="ExternalInput")
    hai = nc.dram_tensor("hai", [128, MT, HID + 1], BF16, kind="ExternalInput")
    uo = nc.dram_tensor("uo", [QB, HID], F32, kind="ExternalOutput")
    io = nc.dram_tensor("io", [QB, HID], F32, kind="ExternalOutput")

    with tile.TileContext(nc) as tc:
        with (
            tc.tile_pool(name="etp", bufs=2) as etp,
            tc.tile_pool(name="ha", bufs=2) as hap,
            tc.tile_pool(name="ob", bufs=4) as obp,
            tc.tile_pool(name="aps", bufs=1, space="PSUM") as aps,
        ):
            for side, (ha_d, out_d) in enumerate(((hau, uo), (hai, io))):
                et_sb = etp.tile([128, MT, QB], BF16, tag="et")
                CH = MT // 4
                for ch in range(4):
                    nc.sync.dma_start(
                        out=et_sb[:, ch * CH:(ch + 1) * CH, :],
                        in_=et.ap()[side, ch * CH:(ch + 1) * CH].rearrange("m p q -> p m q"))
                ha_sb = hap.tile([128, MT, HID + 1], BF16, tag="ha")
                nc.sync.dma_start(out=ha_sb[:], in_=ha_d.ap())

                att_ps = [aps.tile([128, HID + 1], F32, tag=f"att{qt}", name=f"att{qt}_{side}")
                          for qt in range(QT)]
                for m in range(MT):
                    for qt in range(QT):
                        nc.tensor.matmul(
                            att_ps[qt][:],
                            lhsT=et_sb[:, m, qt * 128:(qt + 1) * 128],
                            rhs=ha_sb[:, m, :],
                            start=(m == 0), stop=(m == MT - 1),
                            skip_group_check=True,
                        )
                for qt in range(QT):
                    recip = obp.tile([128, 1], F32, tag="recip")
                    nc.vector.reciprocal(recip[:], att_ps[qt][:, HID:HID + 1])
                    o_sb = obp.tile([128, HID], F32, tag="osb")
                    nc.scalar.activation(o_sb[:], att_ps[qt][:, 0:HID], AF.Relu,
                                         scale=recip[:, 0:1])
                    nc.sync.dma_start(out=out_d.ap()[qt * 128:(qt + 1) * 128, :], in_=o_sb[:])
    nc.compile()
    return nc


_CACHE = {}


def _programs():
    if "k1" not in _CACHE:
        _CACHE["k1"] = _build_k1()
        _CACHE["k2"] = _build_k2()
    return _CACHE["k1"], _CACHE["k2"]


def _arr(x, dt):
    return np.ascontiguousarray(np.asarray(x), dtype=dt)


def _wrap16(a):
    # flat int list -> [128, n/16] int16: index i at partition i%16, slot
    # i//16, replicated for the 8 Q7 cores
    a = np.asarray(a)
    return np.tile(a.reshape(-1, 16).T, (8, 1)).astype(np.int16)


def _cmajor(adj_blk):
    # [UB, DEG] -> flat vals[i], i = (t*DEG+j)*128 + p, user = t*128+p
    return adj_blk.reshape(NT, 128, DEG).transpose(0, 2, 1).reshape(-1)


def _stack_w(w):
    import ml_dtypes
    w4 = w.reshape(DEG, 2, D, HID)
    wa = np.ascontiguousarray(w4[:, 0].reshape(DEG * D, HID).astype(ml_dtypes.bfloat16))
    wb = np.ascontiguousarray(w4[:, 1].reshape(DEG * D, HID).astype(ml_dtypes.bfloat16))
    return wa, wb


def _aug_tiled(h):
    import ml_dtypes
    ha = np.concatenate([h, np.ones((NU, 1), np.float32)], axis=1)
    ha = ha.reshape(MT, 128, HID + 1).transpose(1, 0, 2)
    return np.ascontiguousarray(ha.astype(ml_dtypes.bfloat16))


def kernel(review_vecs, user_vecs, item_vecs, user_weights, item_weights,
           adj0, adj1, adj2, adj3, _profile=None):
    rev = _arr(review_vecs, np.float32)
    uv = _arr(user_vecs, np.float32)
    iv = _arr(item_vecs, np.float32)
    wu = _arr(user_weights, np.float32)
    wi = _arr(item_weights, np.float32)
    a0, a1, a2, a3 = (np.asarray(a).astype(np.int64) for a in (adj0, adj1, adj2, adj3))

    wa_u, wb_u = _stack_w(wu)
    wa_i, wb_i = _stack_w(wi)
    import ml_dtypes
    uvt = np.ascontiguousarray(np.concatenate([uv.T, uv.T], 0).astype(np.float16))
    ivt = np.ascontiguousarray(np.concatenate([iv.T, iv.T], 0).astype(np.float16))

    k1, k2 = _programs()
    cores = list(range(N_CORES))

    in_maps1 = []
    for c in cores:
        bidx = np.zeros((2, 128, NG // 16), np.int16)
        iidx = np.zeros((2, 128, NG // 16), np.int16)
        selm = np.zeros((2, 128, NSLOT, BLK), np.int8)
        for side, (a_rev, a_oth) in enumerate(((a0, a1), (a2, a3))):
            rvals = _cmajor(a_rev[c * UB:(c + 1) * UB])
            ovals = _cmajor(a_oth[c * UB:(c + 1) * UB])
            bidx[side] = _wrap16(rvals // BLK)
            iidx[side] = _wrap16(ovals)
            sel = (rvals % BLK).reshape(NSLOT, 128).T  # [p, c]
            for b in range(BLK):
                selm[side, :, :, b] = (sel == b)
        in_maps1.append({
            "rev": rev, "usert": uv, "itemt": iv,
            "wa_u": wa_u, "wb_u": wb_u, "wa_i": wa_i, "wb_i": wb_i,
            "bidx": bidx, "selm": selm, "iidx": iidx,
            "vtu": uvt, "vtuq": np.ascontiguousarray(uvt[:, c * QB:(c + 1) * QB]),
            "vti": ivt, "vtiq": np.ascontiguousarray(ivt[:, c * QB:(c + 1) * QB]),
        })
    r1 = run_bass_kernel_spmd(k1, in_maps1, core_ids=cores, trace=_profile is not None)
    h_user = np.concatenate([r1.results[c]["hu"] for c in cores], axis=0)
    h_item = np.concatenate([r1.results[c]["hi"] for c in cores], axis=0)

    hau = _aug_tiled(h_user)
    hai = _aug_tiled(h_item)
    in_maps2 = [{
        "et": r1.results[c]["et"], "hau": hau, "hai": hai,
    } for c in cores]
    r2 = run_bass_kernel_spmd(k2, in_maps2, core_ids=cores, trace=_profile is not None)

    user_out = np.concatenate([r2.results[c]["uo"] for c in cores], axis=0)
    item_out = np.concatenate([r2.results[c]["io"] for c in cores], axis=0)

    if _profile is not None:
        _profile["k1"] = r1
        _profile["k2"] = r2
    return user_out, item_out
